# revision 12
# baseline (speedup 1.0000x reference)
"""Trainium2 Bass kernel for nn_Block_8985071583801.

Pipeline per core (1 batch element of 8, data-parallel over batch):
  PPM pool branch -> concat xc[1024,1024] -> in_proj -> causal conv1d+silu
  -> x_proj -> dt_proj+softplus -> selective scan (tensor_tensor_scan)
  -> gate -> out_proj -> 3x3 conv FFN -> bn/relu6 -> fc1+gelu -> fc2.

Layout: channels on partitions, sequence t (=32*32) on the free dim.
All heavy matmuls in bf16 (fp32 PSUM accumulate); scan state fp32 inside
the DVE scan op with bf16 operands (validated to ~2e-5 rel err).
"""

import os
import sys
from contextlib import ExitStack

for _p in ("/opt/trn_rl_repo",):
    if os.path.isdir(_p) and _p not in sys.path:
        sys.path.insert(0, _p)

import numpy as np
import ml_dtypes

BF = ml_dtypes.bfloat16
F32 = np.float32

IN_CHS = 512
DIM = 128
D_MODEL = 1024
D_INNER = 2048
D_STATE = 16
D_CONV = 4
DT_RANK = 64
POOL_SCALES = [1, 5, 9, 13]
B = 8
H = 32
W = 32
L = H * W
NT = D_INNER // 128  # 16 d-tiles
N_CORES = 8
PAD = 3  # conv1d left pad


def _pool_mat(in_size, out_size):
    M = np.zeros((out_size, in_size), np.float32)
    for i in range(out_size):
        s = int(np.floor(i * in_size / out_size))
        e = int(np.ceil((i + 1) * in_size / out_size))
        M[i, s:e] = 1.0 / (e - s)
    return M


def _bilinear_mat(p, out=32):
    """jax.image.resize(method='bilinear') upsample matrix R[out, p]."""
    R = np.zeros((out, p), np.float32)
    for y in range(out):
        c = (y + 0.5) * p / out - 0.5
        f = int(np.floor(c))
        w = c - f
        lo = min(max(f, 0), p - 1)
        hi = min(max(f + 1, 0), p - 1)
        R[y, lo] += 1.0 - w
        R[y, hi] += w
    return R


def _prep_host(inputs):
    """All weight transposes/packs in numpy. Returns dict name->np.ndarray."""
    t = {}
    t["wpool_t"] = np.ascontiguousarray(
        np.concatenate([inputs["w_pool"][i].T for i in range(4)], axis=1)
    ).astype(BF)  # [512, 4*128]
    t["pool_bn_s"] = np.ascontiguousarray(inputs["pool_bn_scale"].T).astype(F32)  # [128,4]
    t["pool_bn_b"] = np.ascontiguousarray(inputs["pool_bn_bias"].T).astype(F32)
    for p in POOL_SCALES[1:]:
        Mh = _pool_mat(H, p)
        Mw = _pool_mat(W, p)
        # bin areas (sum -> mean scale), flattened [p*p], broadcast to 128 rows
        ah = 1.0 / np.array([np.count_nonzero(Mh[i]) for i in range(p)], np.float32)
        aw = 1.0 / np.array([np.count_nonzero(Mw[i]) for i in range(p)], np.float32)
        sc = np.outer(ah, aw).reshape(1, p * p)
        t[f"scpl_{p}"] = np.broadcast_to(sc, (128, p * p)).astype(BF).copy()
        Rh = _bilinear_mat(p, H)
        Rw = _bilinear_mat(p, W)
        W2 = np.kron(Rh, Rw).T  # [p*p, 1024]
        t[f"w2_{p}"] = np.ascontiguousarray(W2).astype(BF)
        # bin boundaries for the reduce-based pooling
        t[f"bins_{p}"] = None  # host-side only
    t["in_proj_wT"] = np.ascontiguousarray(inputs["in_proj_w"].T).astype(BF)  # [1024,4096]
    cw = inputs["conv1d_w"].reshape(NT, 128, D_CONV).transpose(1, 0, 2).reshape(128, NT * D_CONV)
    t["conv_w"] = np.ascontiguousarray(cw).astype(F32)  # [128, 64]
    t["conv_b"] = np.ascontiguousarray(inputs["conv1d_b"].reshape(NT, 128).T).astype(F32)
    t["x_proj_wT"] = np.ascontiguousarray(inputs["x_proj_w"].T).astype(BF)  # [2048, 96]
    t["dt_proj_wT"] = np.ascontiguousarray(inputs["dt_proj_w"].T).astype(BF)  # [64, 2048]
    t["dt_bias"] = np.ascontiguousarray(inputs["dt_proj_b"].reshape(NT, 128).T).astype(F32)
    A = -np.exp(inputs["A_log"].astype(np.float64)).astype(F32)  # [2048, 16]
    t["A_sb"] = np.ascontiguousarray(
        A.reshape(NT, 128, D_STATE).transpose(1, 0, 2).reshape(128, NT * D_STATE)
    ).astype(F32)  # [128, 256]
    t["D_sb"] = np.ascontiguousarray(inputs["D_param"].reshape(NT, 128).T).astype(F32)
    t["out_proj_wT"] = np.ascontiguousarray(inputs["out_proj_w"].T).astype(BF)  # [2048, 1024]
    fw = inputs["ffn_conv_w"]  # [1024 o, 1024 c, 3, 3]
    t["ffn_wT"] = np.ascontiguousarray(
        np.stack([fw[:, :, ky, kx].T for ky in range(3) for kx in range(3)])
    ).astype(BF)  # [9, 1024 c, 1024 o]
    t["ffn_bn_s"] = np.ascontiguousarray(inputs["ffn_bn_scale"].reshape(8, 128).T).astype(F32)
    t["ffn_bn_b"] = np.ascontiguousarray(inputs["ffn_bn_bias"].reshape(8, 128).T).astype(F32)
    t["fc1_wT"] = np.ascontiguousarray(inputs["fc1_w"].T).astype(BF)  # [1024, 512]
    t["fc2_wT"] = np.ascontiguousarray(inputs["fc2_w"].T).astype(BF)  # [512, 128]
    t["ident"] = np.eye(128, dtype=np.float32).astype(BF)
    return t


def _bin_ranges(in_size, p):
    return [
        (int(np.floor(i * in_size / p)), int(np.ceil((i + 1) * in_size / p)))
        for i in range(p)
    ]


def build_program(debug_taps=False):
    import concourse.bass as bass
    from concourse import bacc, mybir, tile

    fp32 = mybir.dt.float32
    bf16 = mybir.dt.bfloat16
    AF = mybir.ActivationFunctionType
    OP = mybir.AluOpType

    nc = bacc.Bacc("TRN2", target_bir_lowering=False, debug=False,
                   enable_asserts=False)

    di = {}  # dram inputs

    def din(name, shape, dt):
        di[name] = nc.dram_tensor(name, list(shape), dt, kind="ExternalInput").ap()

    din("xin", (IN_CHS, L), bf16)
    din("wpool_t", (IN_CHS, 4 * DIM), bf16)
    din("pool_bn_s", (128, 4), fp32)
    din("pool_bn_b", (128, 4), fp32)
    for p in POOL_SCALES[1:]:
        din(f"scpl_{p}", (128, p * p), bf16)
        din(f"w2_{p}", (p * p, 1024), bf16)
    din("in_proj_wT", (D_MODEL, 2 * D_INNER), bf16)
    din("conv_w", (128, NT * D_CONV), fp32)
    din("conv_b", (128, NT), fp32)
    din("x_proj_wT", (D_INNER, 96), bf16)
    din("dt_proj_wT", (DT_RANK, D_INNER), bf16)
    din("dt_bias", (128, NT), fp32)
    din("A_sb", (128, NT * D_STATE), fp32)
    din("D_sb", (128, NT), fp32)
    din("out_proj_wT", (D_INNER, D_MODEL), bf16)
    din("ffn_wT", (9, D_MODEL, D_MODEL), bf16)
    din("ffn_bn_s", (128, 8), fp32)
    din("ffn_bn_b", (128, 8), fp32)
    din("fc1_wT", (D_MODEL, 512), bf16)
    din("fc2_wT", (512, 128), bf16)
    din("ident", (128, 128), bf16)

    out_dram = nc.dram_tensor("out", [128, L], fp32, kind="ExternalOutput").ap()

    taps = {}
    if debug_taps:
        for nm, shape in (
            ("t_xc", (D_MODEL, L)),
            ("t_xm", (D_INNER, L)),
            ("t_xmc", (D_INNER, L)),
            ("t_xdbl", (96, L)),
            ("t_dt", (D_INNER, L)),
            ("t_yg", (D_INNER, L)),
            ("t_conv", (D_MODEL, L)),
        ):
            taps[nm] = nc.dram_tensor(nm, list(shape), fp32, kind="ExternalOutput").ap()

    with tile.TileContext(nc) as tc, ExitStack() as ctx:
        ve = nc.vector
        se = nc.scalar
        ge = nc.gpsimd
        te = nc.tensor

        # ---------------- persistent pools ----------------
        cst = ctx.enter_context(tc.tile_pool(name="cst", bufs=1))

        def cdma(name, shape, dt):
            t_ = cst.tile(list(shape), dt, tag=name)
            nc.sync.dma_start(t_[:], di[name][:])
            return t_

        ident = cdma("ident", (128, 128), bf16)
        conv_w = cdma("conv_w", (128, NT * D_CONV), fp32)
        conv_b = cdma("conv_b", (128, NT), fp32)
        dt_bias = cdma("dt_bias", (128, NT), fp32)
        A_sb = cdma("A_sb", (128, NT * D_STATE), fp32)
        D_sb = cdma("D_sb", (128, NT), fp32)
        pbs = cdma("pool_bn_s", (128, 4), fp32)
        pbb = cdma("pool_bn_b", (128, 4), fp32)
        fbs = cdma("ffn_bn_s", (128, 8), fp32)
        fbb = cdma("ffn_bn_b", (128, 8), fp32)
        dt_proj_wT = cdma("dt_proj_wT", (DT_RANK, D_INNER), bf16)

        xmcp = ctx.enter_context(tc.tile_pool(name="xmcp", bufs=NT))
        zygp = ctx.enter_context(tc.tile_pool(name="zygp", bufs=NT + 1))

        # xc: 8 tiles of [128, 1024] bf16 (c-part). rows: 0..511 x, 512..639
        # pool0, 640..1023 pools 1-3
        pxc = ctx.enter_context(ExitStack())
        xcp = pxc.enter_context(tc.tile_pool(name="xcp", bufs=9))
        ones = xcp.tile([128, L], bf16, tag="ones")
        ve.memset(ones[:], 1.0)
        xc = [xcp.tile([128, L], bf16, tag="xc", name=f"xc{i}") for i in range(8)]
        for k in range(4):
            nc.sync.dma_start(xc[k][:], di["xin"][k * 128:(k + 1) * 128, :])

        def relu6(dst, src):
            ve.tensor_scalar(dst, src, 0.0, 6.0, OP.max, OP.min)

        # ================= PHASE 1: pool branch =================
        with ExitStack() as p1:
            ps = p1.enter_context(tc.tile_pool(name="ps1", bufs=4, space="PSUM"))
            pst = p1.enter_context(tc.tile_pool(name="ps1t", bufs=2, space="PSUM"))
            wp = p1.enter_context(tc.tile_pool(name="wp1", bufs=6))
            tp = p1.enter_context(tc.tile_pool(name="tp1", bufs=2))

            wpt = [wp.tile([128, 128], bf16, tag="wpool", name=f"wpt{i}") for i in range(4 * 4)]
            for i in range(4):
                for k in range(4):
                    nc.sync.dma_start(
                        wpt[i * 4 + k][:],
                        di["wpool_t"][k * 128:(k + 1) * 128, i * 128:(i + 1) * 128],
                    )

            # ---- pool0: 1x1 conv over full res, bn, relu6, mean, broadcast
            t0 = tp.tile([128, L], fp32, tag="t0", bufs=1)
            for th in range(2):
                acc = ps.tile([128, 512], fp32, tag="mm")
                for k in range(4):
                    te.matmul(acc[:], wpt[k][:], xc[k][:, th * 512:(th + 1) * 512],
                              start=(k == 0), stop=(k == 3))
                se.activation(t0[:, th * 512:(th + 1) * 512], acc[:], AF.Identity,
                              bias=pbb[:, 0:1], scale=pbs[:, 0:1])
            t0b = tp.tile([128, L], fp32, tag="t0b", bufs=1)
            relu6(t0b[:], t0[:])
            mean = tp.tile([128, 1], fp32, tag="mean")
            ve.tensor_reduce(mean[:], t0b[:], mybir.AxisListType.X, OP.add)
            means = tp.tile([128, 1], fp32, tag="means")
            ve.tensor_scalar_mul(means[:], mean[:], 1.0 / L)
            se.activation(xc[4][:], ones[:], AF.Copy, scale=means[:, 0:1])

            # ---- pools 1..3
            for i, p in enumerate(POOL_SCALES[1:], start=1):
                hb = _bin_ranges(H, p)
                wb = _bin_ranges(W, p)
                pp = p * p
                scpl = tp.tile([128, pp], bf16, tag="scpl")
                nc.sync.dma_start(scpl[:], di[f"scpl_{p}"][:])
                nk = (pp + 127) // 128
                w2t = [wp.tile([128, 1024], bf16, tag="w2", name=f"w2t{kk}") for kk in range(nk)]
                for kk in range(nk):
                    r0 = kk * 128
                    r1 = min(pp, r0 + 128)
                    nc.sync.dma_start(w2t[kk][0:r1 - r0, :], di[f"w2_{p}"][r0:r1, :])

                conv_ps = pst.tile([128, pp], fp32, tag="cps")
                for k in range(4):
                    # pool w then h via strided reduces, per c-tile
                    pw = tp.tile([128, 32 * p], fp32, tag="pw")  # [h, q]
                    xv = xc[k][:].rearrange("c (h w) -> c h w", w=W)
                    pwv = pw[:].rearrange("c (h q) -> c h q", q=p)
                    for qi, (s, e) in enumerate(wb):
                        ve.tensor_reduce(pwv[:, :, qi:qi + 1], xv[:, :, s:e],
                                         mybir.AxisListType.X, OP.add)
                    pooled = tp.tile([128, pp], fp32, tag="pooled")
                    pldv = pooled[:].rearrange("c (ph q) -> c ph q", q=p)
                    pwT = pw[:].rearrange("c (h q) -> c q h", q=p)  # strided view
                    for pi, (s, e) in enumerate(hb):
                        ve.tensor_reduce(pldv[:, pi, :].unsqueeze(1), pwT[:, :, s:e],
                                         mybir.AxisListType.X, OP.add)
                    pooled_s = tp.tile([128, pp], bf16, tag="pooled_s")
                    ve.tensor_tensor(pooled_s[:], pooled[:], scpl[:], OP.mult)
                    te.matmul(conv_ps[:], wpt[i * 4 + k][:], pooled_s[:],
                              start=(k == 0), stop=(k == 3))
                tbn = tp.tile([128, pp], fp32, tag="tbn")
                se.activation(tbn[:], conv_ps[:], AF.Identity,
                              bias=pbb[:, i:i + 1], scale=pbs[:, i:i + 1])
                tr6 = tp.tile([128, pp], bf16, tag="tr6")
                relu6(tr6[:], tbn[:])
                # transpose [128, pp] -> [pp, 128]
                tT = [tp.tile([128, 128], bf16, tag="tT", name=f"tT{kk}") for kk in range(nk)]
                for kk in range(nk):
                    r0 = kk * 128
                    r1 = min(pp, r0 + 128)
                    tps_ = pst.tile([128, 128], bf16, tag="trp")
                    te.transpose(tps_[0:r1 - r0, :], tr6[:, r0:r1], ident[:])
                    se.activation(tT[kk][0:r1 - r0, :], tps_[0:r1 - r0, :], AF.Copy)
                for th in range(2):
                    rs = ps.tile([128, 512], fp32, tag="mm")
                    for kk in range(nk):
                        r0 = kk * 128
                        r1 = min(pp, r0 + 128)
                        te.matmul(rs[:], tT[kk][0:r1 - r0, :],
                                  w2t[kk][0:r1 - r0, th * 512:(th + 1) * 512],
                                  start=(kk == 0), stop=(kk == nk - 1))
                    se.activation(xc[4 + i][:, th * 512:(th + 1) * 512], rs[:], AF.Copy)

        if debug_taps:
            for k in range(8):
                cp = cst.tile([128, L], fp32, tag="dbgcp")
                se.activation(cp[:], xc[k][:], AF.Copy)
                nc.sync.dma_start(taps["t_xc"][k * 128:(k + 1) * 128, :], cp[:])

        # ============ PHASE 2+3: in_proj, conv1d+silu ============
        xmc = [None] * NT
        z = [None] * NT

        with ExitStack() as p2:
            ps = p2.enter_context(tc.tile_pool(name="ps2", bufs=4, space="PSUM"))
            wp = p2.enter_context(tc.tile_pool(name="wp2", bufs=10))
            xmp = p2.enter_context(tc.tile_pool(name="xmp", bufs=4))
            ac = p2.enter_context(tc.tile_pool(name="ac2", bufs=4))

            for m in range(2 * NT):  # 0..15 xm, 16..31 z
                wt = [wp.tile([128, 128], bf16, tag="w_in", name=f"win{k}") for k in range(8)]
                for k in range(8):
                    nc.sync.dma_start(
                        wt[k][:],
                        di["in_proj_wT"][k * 128:(k + 1) * 128, m * 128:(m + 1) * 128],
                    )
                if m < NT:
                    xm_t = xmp.tile([128, L + PAD], bf16, tag="xm")
                    ve.memset(xm_t[:, 0:PAD], 0.0)
                    dst = xm_t
                    off = PAD
                else:
                    z[m - NT] = zygp.tile([128, L], bf16, tag="zyg", name=f"z{m}")
                    dst = z[m - NT]
                    off = 0
                for th in range(2):
                    acc = ps.tile([128, 512], fp32, tag="mm")
                    for k in range(8):
                        te.matmul(acc[:], wt[k][:], xc[k][:, th * 512:(th + 1) * 512],
                                  start=(k == 0), stop=(k == 7))
                    # z rows get silu applied here (Silu table active in this
                    # phase) so the scan phase only needs the Exp/Ln table.
                    se.activation(dst[:, off + th * 512: off + (th + 1) * 512],
                                  acc[:], AF.Copy if m < NT else AF.Silu)
                if m < NT:
                    # causal depthwise conv1d + silu
                    a0 = ac.tile([128, L], bf16, tag="cacc")
                    a1 = ac.tile([128, L], bf16, tag="cacc")
                    ve.tensor_scalar_mul(a0[:], xm_t[:, 0:L], conv_w[:, m * 4:m * 4 + 1])
                    ve.scalar_tensor_tensor(a1[:], xm_t[:, 1:1 + L],
                                            conv_w[:, m * 4 + 1:m * 4 + 2], a0[:],
                                            OP.mult, OP.add)
                    ve.scalar_tensor_tensor(a0[:], xm_t[:, 2:2 + L],
                                            conv_w[:, m * 4 + 2:m * 4 + 3], a1[:],
                                            OP.mult, OP.add)
                    ve.scalar_tensor_tensor(a1[:], xm_t[:, 3:3 + L],
                                            conv_w[:, m * 4 + 3:m * 4 + 4], a0[:],
                                            OP.mult, OP.add)
                    xmc[m] = xmcp.tile([128, L], bf16, tag="xmc", name=f"xmc{m}")
                    se.activation(xmc[m][:], a1[:], AF.Silu, bias=conv_b[:, m:m + 1])

        if debug_taps:
            for k in range(NT):
                cp = cst.tile([128, L], fp32, tag="dbgcp")
                se.activation(cp[:], xmc[k][:], AF.Copy)
                nc.sync.dma_start(taps["t_xmc"][k * 128:(k + 1) * 128, :], cp[:])

        pxc.close()  # frees xc + ones

        # ============ PHASE 4: x_proj ============
        xdbl_sb = cst.tile([96, L], bf16, tag="xdbl")
        with ExitStack() as p4:
            psb = p4.enter_context(tc.tile_pool(name="ps4", bufs=2, space="PSUM"))
            wp = p4.enter_context(tc.tile_pool(name="wp4", bufs=4))
            xd_ps = psb.tile([128, L], fp32, tag="xd")
            for k in range(NT):
                wt = wp.tile([128, 96], bf16, tag="w_xp")
                nc.sync.dma_start(wt[:], di["x_proj_wT"][k * 128:(k + 1) * 128, :])
                for th in range(2):
                    te.matmul(xd_ps[0:96, th * 512:(th + 1) * 512], wt[:],
                              xmc[k][:, th * 512:(th + 1) * 512],
                              start=(k == 0), stop=(k == NT - 1))
            se.activation(xdbl_sb[:], xd_ps[0:96, :], AF.Copy)

        if debug_taps:
            cp = cst.tile([96, L], fp32, tag="dbgcp96")
            se.activation(cp[:], xdbl_sb[:], AF.Copy)
            nc.sync.dma_start(taps["t_xdbl"][:], cp[:])

        # ============ PHASE 5: broadcasts + scan + gate ============
        bcp = ctx.enter_context(tc.tile_pool(name="bcp", bufs=32))
        Bb = [bcp.tile([128, L], bf16, tag="bc", name=f"Bb{n}") for n in range(D_STATE)]
        Cb = [bcp.tile([128, L], bf16, tag="bc", name=f"Cb{n}") for n in range(D_STATE)]
        bsp = ctx.enter_context(tc.tile_pool(name="bsp", bufs=4))
        for n in range(2 * D_STATE):
            dst = Bb[n] if n < D_STATE else Cb[n - D_STATE]
            stg = bsp.tile([1, L], bf16, tag="bstg", name=f"bstg{n}")
            nc.sync.dma_start(stg[0:1, :], xdbl_sb[DT_RANK + n:DT_RANK + n + 1, :])
            ge.partition_broadcast(dst[:], stg[0:1, :])

        yg = [None] * NT
        with ExitStack() as p5:
            psb = p5.enter_context(tc.tile_pool(name="ps5", bufs=2, space="PSUM"))
            sp = p5.enter_context(tc.tile_pool(name="sp5", bufs=2))
            spb = p5.enter_context(tc.tile_pool(name="spb5", bufs=2))
            ysp = p5.enter_context(tc.tile_pool(name="ysp", bufs=1))

            for k in range(NT):
                dt_ps = psb.tile([128, L], fp32, tag="dtps")
                for th in range(2):
                    te.matmul(dt_ps[:, th * 512:(th + 1) * 512],
                              dt_proj_wT[:, k * 128:(k + 1) * 128],
                              xdbl_sb[0:DT_RANK, th * 512:(th + 1) * 512],
                              start=True, stop=True)
                dt_e = sp.tile([128, L], fp32, tag="dte", bufs=2)
                se.activation(dt_e[:], dt_ps[:], AF.Exp, bias=dt_bias[:, k:k + 1])
                dt_t = sp.tile([128, L], bf16, tag="dt")
                se.activation(dt_t[:], dt_e[:], AF.Ln, bias=1.0)
                if debug_taps:
                    cp = cst.tile([128, L], fp32, tag="dbgcp")
                    se.activation(cp[:], dt_t[:], AF.Copy)
                    nc.sync.dma_start(taps["t_dt"][k * 128:(k + 1) * 128, :], cp[:])
                w_t = sp.tile([128, L], bf16, tag="wt", bufs=1)
                ve.tensor_tensor(w_t[:], dt_t[:], xmc[k][:], OP.mult)
                ys = ysp.tile([128, D_STATE * L], bf16, tag="ys")
                for n in range(D_STATE):
                    a_t = spb.tile([128, L], bf16, tag="a")
                    se.activation(a_t[:], dt_t[:], AF.Exp,
                                  scale=A_sb[:, k * D_STATE + n:k * D_STATE + n + 1])
                    b_t = spb.tile([128, L], bf16, tag="b")
                    ve.tensor_tensor(b_t[:], w_t[:], Bb[n][:], OP.mult)
                    h_t = spb.tile([128, L], bf16, tag="h")
                    ve.tensor_tensor_scan(h_t[:], a_t[:], b_t[:], 0.0, OP.mult, OP.add)
                    ve.tensor_tensor(ys[:, n * L:(n + 1) * L], h_t[:], Cb[n][:], OP.mult)
                # in-place pairwise tree sum over the 16 slabs
                stride = 1
                cnt = D_STATE
                while cnt > 1:
                    for i in range(cnt // 2):
                        ve.tensor_tensor(
                            ys[:, (2 * i) * stride * L:((2 * i) * stride + 1) * L],
                            ys[:, (2 * i) * stride * L:((2 * i) * stride + 1) * L],
                            ys[:, (2 * i + 1) * stride * L:((2 * i + 1) * stride + 1) * L],
                            OP.add)
                    stride *= 2
                    cnt //= 2
                yfull = sp.tile([128, L], bf16, tag="yfull", bufs=1)
                ve.scalar_tensor_tensor(yfull[:], xmc[k][:], D_sb[:, k:k + 1],
                                        ys[:, 0:L], OP.mult, OP.add)
                yg[k] = zygp.tile([128, L], bf16, tag="zyg", name=f"yg{k}")
                ve.tensor_tensor(yg[k][:], yfull[:], z[k][:], OP.mult)

        if debug_taps:
            for k in range(NT):
                cp = cst.tile([128, L], fp32, tag="dbgcp")
                se.activation(cp[:], yg[k][:], AF.Copy)
                nc.sync.dma_start(taps["t_yg"][k * 128:(k + 1) * 128, :], cp[:])

        # ============ PHASE 7: out_proj -> padded conv input ============
        cvp = ctx.enter_context(tc.tile_pool(name="cvp", bufs=8))
        convpad = [cvp.tile([128, 34 * 34], bf16, tag="cvpad", name=f"cvpad{m}") for m in range(8)]
        for m in range(8):
            ve.memset(convpad[m][:], 0.0)

        with ExitStack() as p7:
            ps = p7.enter_context(tc.tile_pool(name="ps7", bufs=4, space="PSUM"))
            wp = p7.enter_context(tc.tile_pool(name="wp7", bufs=20))
            for m in range(8):
                wt = [wp.tile([128, 128], bf16, tag="w_op", name=f"wop{k}") for k in range(NT)]
                for k in range(NT):
                    nc.sync.dma_start(
                        wt[k][:],
                        di["out_proj_wT"][k * 128:(k + 1) * 128, m * 128:(m + 1) * 128])
                for th in range(2):
                    acc = ps.tile([128, 512], fp32, tag="mm")
                    for k in range(NT):
                        te.matmul(acc[:], wt[k][:], yg[k][:, th * 512:(th + 1) * 512],
                                  start=(k == 0), stop=(k == NT - 1))
                    dstv = convpad[m][:].rearrange("c (h w) -> c h w", w=34)
                    se.activation(
                        dstv[:, 1 + th * 16:1 + (th + 1) * 16, 1:33],
                        acc[:].rearrange("c (h w) -> c h w", w=32), AF.Copy)

        # ============ PHASE 8: FFN 3x3 conv + bn + relu6 ============
        t2p = ctx.enter_context(tc.tile_pool(name="t2p", bufs=8))
        t2 = [t2p.tile([128, L], bf16, tag="t2", name=f"t2_{m}") for m in range(8)]
        with ExitStack() as p8:
            ps = p8.enter_context(tc.tile_pool(name="ps8", bufs=4, space="PSUM"))
            wp = p8.enter_context(tc.tile_pool(name="wp8", bufs=76))
            tmp = p8.enter_context(tc.tile_pool(name="tmp8", bufs=2))
            for m in range(8):
                wt = {}
                for tap in range(9):
                    for k in range(8):
                        wt[tap, k] = wp.tile([128, 128], bf16, tag="w_ffn", name=f"wffn{tap}_{k}")
                        nc.sync.dma_start(
                            wt[tap, k][:],
                            di["ffn_wT"][tap, k * 128:(k + 1) * 128,
                                         m * 128:(m + 1) * 128])
                for th in range(2):
                    acc = ps.tile([128, 512], fp32, tag="mm")
                    first = True
                    for tap in range(9):
                        ky, kx = tap // 3, tap % 3
                        br = th * 16 + ky
                        for k in range(8):
                            rhs = convpad[k][:].rearrange(
                                "c (h w) -> c h w", w=34)[:, br:br + 16, kx:kx + 32]
                            te.matmul(acc[:], wt[tap, k][:], rhs,
                                      start=first, stop=(tap == 8 and k == 7))
                            first = False
                    tb = tmp.tile([128, 512], fp32, tag="tbn8")
                    se.activation(tb[:], acc[:], AF.Identity,
                                  bias=fbb[:, m:m + 1], scale=fbs[:, m:m + 1])
                    relu6(t2[m][:, th * 512:(th + 1) * 512], tb[:])

        if debug_taps:
            for k in range(8):
                cp = cst.tile([128, L], fp32, tag="dbgcp")
                se.activation(cp[:], t2[k][:], AF.Copy)
                nc.sync.dma_start(taps["t_conv"][k * 128:(k + 1) * 128, :], cp[:])

        # ============ PHASE 9/10: fc1+gelu, fc2 ============
        with ExitStack() as p9:
            ps = p9.enter_context(tc.tile_pool(name="ps9", bufs=4, space="PSUM"))
            wp = p9.enter_context(tc.tile_pool(name="wp9", bufs=10))
            gp = p9.enter_context(tc.tile_pool(name="gp9", bufs=4))
            g = [gp.tile([128, L], bf16, tag="g", name=f"g{i}") for i in range(4)]
            for mo in range(4):
                wt = [wp.tile([128, 128], bf16, tag="w_fc1", name=f"wfc1_{k}") for k in range(8)]
                for k in range(8):
                    nc.sync.dma_start(
                        wt[k][:],
                        di["fc1_wT"][k * 128:(k + 1) * 128, mo * 128:(mo + 1) * 128])
                for th in range(2):
                    acc = ps.tile([128, 512], fp32, tag="mm")
                    for k in range(8):
                        te.matmul(acc[:], wt[k][:], t2[k][:, th * 512:(th + 1) * 512],
                                  start=(k == 0), stop=(k == 7))
                    se.activation(g[mo][:, th * 512:(th + 1) * 512], acc[:], AF.Gelu)
            out_sb = gp.tile([128, L], fp32, tag="outsb", bufs=1)
            wt2 = [wp.tile([128, 128], bf16, tag="w_fc2", name=f"wfc2_{k}") for k in range(4)]
            for k in range(4):
                nc.sync.dma_start(wt2[k][:], di["fc2_wT"][k * 128:(k + 1) * 128, :])
            for th in range(2):
                acc = ps.tile([128, 512], fp32, tag="mm")
                for k in range(4):
                    te.matmul(acc[:], wt2[k][:], g[k][:, th * 512:(th + 1) * 512],
                              start=(k == 0), stop=(k == 3))
                se.activation(out_sb[:, th * 512:(th + 1) * 512], acc[:], AF.Copy)
            nc.sync.dma_start(out_dram[:], out_sb[:])

    nc.compile()
    return nc


_PROGRAM_CACHE = {}


def run(inputs, debug_taps=False, trace=False):
    from concourse.bass_utils import run_bass_kernel_spmd

    key = debug_taps
    if key not in _PROGRAM_CACHE:
        _PROGRAM_CACHE[key] = build_program(debug_taps)
    nc = _PROGRAM_CACHE[key]

    host = _prep_host(inputs)
    x = np.asarray(inputs["x"], dtype=np.float32)
    in_maps = []
    for b in range(N_CORES):
        m = {k: v for k, v in host.items() if v is not None and not k.startswith("bins")}
        m["xin"] = np.ascontiguousarray(x[b].reshape(IN_CHS, L)).astype(BF)
        in_maps.append(m)
    res = run_bass_kernel_spmd(nc, in_maps, core_ids=list(range(N_CORES)),
                               trace=trace)
    outs = np.stack([res.results[b]["out"].reshape(128, H, W)
                     for b in range(N_CORES)])
    return outs.astype(np.float32), res


def kernel(**inputs) -> np.ndarray:
    out, _ = run(inputs)
    return out


# revision 33
# speedup vs baseline: 61.1142x; 61.1142x over previous
"""Trainium2 Bass kernel for nn_Block_8985071583801.

Pipeline per core (1 batch element of 8, data-parallel over batch):
  PPM pool branch -> concat xc[1024,1024] -> in_proj -> causal conv1d+silu
  -> x_proj -> dt_proj+softplus -> selective scan (tensor_tensor_scan)
  -> gate -> out_proj -> 3x3 conv FFN -> bn/relu6 -> fc1+gelu -> fc2.

Layout: channels on partitions, sequence t (=32*32) on the free dim.
All heavy matmuls in bf16 (fp32 PSUM accumulate); scan state fp32 inside
the DVE scan op with bf16 operands (validated to ~2e-5 rel err).
"""

import os
import sys
from contextlib import ExitStack

for _p in ("/opt/trn_rl_repo",):
    if os.path.isdir(_p) and _p not in sys.path:
        sys.path.insert(0, _p)

import numpy as np
import ml_dtypes

BF = ml_dtypes.bfloat16
F32 = np.float32

IN_CHS = 512
DIM = 128
D_MODEL = 1024
D_INNER = 2048
D_STATE = 16
D_CONV = 4
DT_RANK = 64
POOL_SCALES = [1, 5, 9, 13]
B = 8
H = 32
W = 32
L = H * W
NT = D_INNER // 128  # 16 d-tiles
N_CORES = 8
PAD = 3  # conv1d left pad


def _pool_mat(in_size, out_size):
    M = np.zeros((out_size, in_size), np.float32)
    for i in range(out_size):
        s = int(np.floor(i * in_size / out_size))
        e = int(np.ceil((i + 1) * in_size / out_size))
        M[i, s:e] = 1.0 / (e - s)
    return M


def _bilinear_mat(p, out=32):
    """jax.image.resize(method='bilinear') upsample matrix R[out, p]."""
    R = np.zeros((out, p), np.float32)
    for y in range(out):
        c = (y + 0.5) * p / out - 0.5
        f = int(np.floor(c))
        w = c - f
        lo = min(max(f, 0), p - 1)
        hi = min(max(f + 1, 0), p - 1)
        R[y, lo] += 1.0 - w
        R[y, hi] += w
    return R


def _prep_host(inputs):
    """All weight transposes/packs in numpy. Returns dict name->np.ndarray."""
    t = {}
    wpt_ = np.concatenate([inputs["w_pool"][i].T for i in range(4)], axis=1)  # [512, 512]
    t["wpool_pk"] = np.ascontiguousarray(
        wpt_.reshape(4, 128, 512).transpose(1, 0, 2).reshape(128, 2048)
    ).astype(BF)
    t["pool_bn_s"] = np.ascontiguousarray(inputs["pool_bn_scale"].T).astype(F32)  # [128,4]
    t["pool_bn_b"] = np.ascontiguousarray(inputs["pool_bn_bias"].T).astype(F32)
    for p in POOL_SCALES[1:]:
        Mh = _pool_mat(H, p)
        Mw = _pool_mat(W, p)
        # bin areas (sum -> mean scale), flattened [p*p], broadcast to 128 rows
        ah = 1.0 / np.array([np.count_nonzero(Mh[i]) for i in range(p)], np.float32)
        aw = 1.0 / np.array([np.count_nonzero(Mw[i]) for i in range(p)], np.float32)
        sc = np.outer(ah, aw).reshape(1, p * p)
        t[f"scpl_{p}"] = np.broadcast_to(sc, (128, p * p)).astype(BF).copy()
        Rh = _bilinear_mat(p, H)
        Rw = _bilinear_mat(p, W)
        W2 = np.kron(Rh, Rw).T  # [p*p, 1024]
        t[f"w2_{p}"] = np.ascontiguousarray(W2).astype(BF)
        # bin boundaries for the reduce-based pooling
        t[f"bins_{p}"] = None  # host-side only
    Wt = inputs["in_proj_w"].T  # [1024, 4096]
    t["in_proj_pk"] = np.ascontiguousarray(
        Wt.reshape(8, 128, 32, 128).transpose(2, 1, 0, 3).reshape(32, 128, 1024)
    ).astype(BF)
    cw = inputs["conv1d_w"].reshape(NT, 128, D_CONV).transpose(1, 0, 2).reshape(128, NT * D_CONV)
    t["conv_w"] = np.ascontiguousarray(cw).astype(F32)  # [128, 64]
    t["conv_b"] = np.ascontiguousarray(inputs["conv1d_b"].reshape(NT, 128).T).astype(F32)
    t["x_proj_pk"] = np.ascontiguousarray(
        inputs["x_proj_w"].T.reshape(16, 128, 96).transpose(1, 0, 2).reshape(128, 16 * 96)
    ).astype(BF)
    t["dt_proj_wT"] = np.ascontiguousarray(inputs["dt_proj_w"].T).astype(BF)  # [64, 2048]
    t["dt_bias"] = np.ascontiguousarray(inputs["dt_proj_b"].reshape(NT, 128).T).astype(F32)
    A = -np.exp(inputs["A_log"].astype(np.float64)).astype(F32)  # [2048, 16]
    t["A_sb"] = np.ascontiguousarray(
        A.reshape(NT, 128, D_STATE).transpose(1, 0, 2).reshape(128, NT * D_STATE)
    ).astype(F32)  # [128, 256]
    t["D_sb"] = np.ascontiguousarray(inputs["D_param"].reshape(NT, 128).T).astype(F32)
    t["out_proj_pk"] = np.ascontiguousarray(
        inputs["out_proj_w"].T.reshape(16, 128, 8, 128).transpose(2, 1, 0, 3).reshape(8, 128, 2048)
    ).astype(BF)
    fw = inputs["ffn_conv_w"]  # [1024 o, 1024 c, 3, 3]
    fstk = np.stack([fw[:, :, ky, kx].T for ky in range(3) for kx in range(3)])
    t["ffn_pk"] = np.ascontiguousarray(
        fstk.reshape(9, 8, 128, 8, 128).transpose(3, 2, 0, 1, 4).reshape(8, 128, 9 * 8 * 128)
    ).astype(BF)
    t["ffn_bn_s"] = np.ascontiguousarray(inputs["ffn_bn_scale"].reshape(8, 128).T).astype(F32)
    t["ffn_bn_b"] = np.ascontiguousarray(inputs["ffn_bn_bias"].reshape(8, 128).T).astype(F32)
    t["fc1_pk"] = np.ascontiguousarray(
        inputs["fc1_w"].T.reshape(8, 128, 4, 128).transpose(2, 1, 0, 3).reshape(4, 128, 1024)
    ).astype(BF)
    t["fc2_pk"] = np.ascontiguousarray(
        inputs["fc2_w"].T.reshape(4, 128, 128).transpose(1, 0, 2).reshape(128, 512)
    ).astype(BF)
    t["ident"] = np.eye(128, dtype=np.float32).astype(BF)
    return t


def _bin_ranges(in_size, p):
    return [
        (int(np.floor(i * in_size / p)), int(np.ceil((i + 1) * in_size / p)))
        for i in range(p)
    ]


def build_program(debug_taps=False, gp_b=0, gp_y=16):
    GP_B = lambda n: n < gp_b
    GP_Y = lambda n: n < gp_y
    import concourse.bass as bass
    from concourse import bacc, mybir, tile

    fp32 = mybir.dt.float32
    bf16 = mybir.dt.bfloat16
    AF = mybir.ActivationFunctionType
    OP = mybir.AluOpType

    nc = bacc.Bacc("TRN2", target_bir_lowering=False, debug=False,
                   enable_asserts=False)

    di = {}  # dram inputs

    def din(name, shape, dt):
        di[name] = nc.dram_tensor(name, list(shape), dt, kind="ExternalInput").ap()

    din("xin", (IN_CHS, L), bf16)
    din("wpool_pk", (128, 2048), bf16)
    din("pool_bn_s", (128, 4), fp32)
    din("pool_bn_b", (128, 4), fp32)
    for p in POOL_SCALES[1:]:
        din(f"scpl_{p}", (128, p * p), bf16)
        din(f"w2_{p}", (p * p, 1024), bf16)
    din("in_proj_pk", (32, 128, 1024), bf16)
    din("conv_w", (128, NT * D_CONV), fp32)
    din("conv_b", (128, NT), fp32)
    din("x_proj_pk", (128, 16 * 96), bf16)
    din("dt_proj_wT", (DT_RANK, D_INNER), bf16)
    din("dt_bias", (128, NT), fp32)
    din("A_sb", (128, NT * D_STATE), fp32)
    din("D_sb", (128, NT), fp32)
    din("out_proj_pk", (8, 128, 2048), bf16)
    din("ffn_pk", (8, 128, 9 * 8 * 128), bf16)
    din("ffn_bn_s", (128, 8), fp32)
    din("ffn_bn_b", (128, 8), fp32)
    din("fc1_pk", (4, 128, 1024), bf16)
    din("fc2_pk", (128, 512), bf16)
    din("ident", (128, 128), bf16)

    out_dram = nc.dram_tensor("out", [128, L], fp32, kind="ExternalOutput").ap()

    taps = {}
    if debug_taps:
        for nm, shape in (
            ("t_xc", (D_MODEL, L)),
            ("t_xm", (D_INNER, L)),
            ("t_xmc", (D_INNER, L)),
            ("t_xdbl", (96, L)),
            ("t_dt", (D_INNER, L)),
            ("t_yg", (D_INNER, L)),
            ("t_conv", (D_MODEL, L)),
        ):
            taps[nm] = nc.dram_tensor(nm, list(shape), fp32, kind="ExternalOutput").ap()

    with tile.TileContext(nc) as tc, ExitStack() as ctx:
        ve = nc.vector
        se = nc.scalar
        ge = nc.gpsimd
        te = nc.tensor

        # ---------------- persistent pools ----------------
        cst = ctx.enter_context(tc.tile_pool(name="cst", bufs=1))

        def cdma(name, shape, dt):
            t_ = cst.tile(list(shape), dt, tag=name)
            nc.sync.dma_start(t_[:], di[name][:])
            return t_

        ident = cdma("ident", (128, 128), bf16)
        conv_w = cdma("conv_w", (128, NT * D_CONV), fp32)
        conv_b = cdma("conv_b", (128, NT), fp32)
        dt_bias = cdma("dt_bias", (128, NT), fp32)
        A_sb = cdma("A_sb", (128, NT * D_STATE), fp32)
        D_sb = cdma("D_sb", (128, NT), fp32)
        pbs = cdma("pool_bn_s", (128, 4), fp32)
        pbb = cdma("pool_bn_b", (128, 4), fp32)
        fbs = cdma("ffn_bn_s", (128, 8), fp32)
        fbb = cdma("ffn_bn_b", (128, 8), fp32)
        dt_proj_wT = cdma("dt_proj_wT", (DT_RANK, D_INNER), bf16)

        zygp = ctx.enter_context(tc.tile_pool(name="zygp", bufs=NT + 1))
        pmid = ctx.enter_context(ExitStack())
        xmcp = pmid.enter_context(tc.tile_pool(name="xmcp", bufs=NT))

        # xc: 8 tiles of [128, 1024] bf16 (c-part). rows: 0..511 x, 512..639
        # pool0, 640..1023 pools 1-3
        pxc = pmid.enter_context(ExitStack())
        xcp = pxc.enter_context(tc.tile_pool(name="xcp", bufs=9))
        ones = xcp.tile([128, L], bf16, tag="ones")
        ve.memset(ones[:], 1.0)
        xc = [xcp.tile([128, L], bf16, tag="xc", name=f"xc{i}") for i in range(8)]
        for k in range(4):
            nc.sync.dma_start(xc[k][:], di["xin"][k * 128:(k + 1) * 128, :])

        def relu6(dst, src):
            ve.tensor_scalar(dst, src, 0.0, 6.0, OP.max, OP.min)

        # ================= PHASE 1: pool branch =================
        with ExitStack() as p1:
            ps = p1.enter_context(tc.tile_pool(name="ps1", bufs=4, space="PSUM"))
            pst = p1.enter_context(tc.tile_pool(name="ps1t", bufs=2, space="PSUM"))
            wp = p1.enter_context(tc.tile_pool(name="wp1", bufs=6))
            tp = p1.enter_context(tc.tile_pool(name="tp1", bufs=2))

            wpan = wp.tile([128, 2048], bf16, tag="wpool")
            nc.sync.dma_start(wpan[:], di["wpool_pk"][:])
            # lhsT for (pool i, ctile k) = wpan[:, k*512 + i*128 :][:128]
            wpt = [wpan[:, k * 512 + i * 128: k * 512 + (i + 1) * 128]
                   for i in range(4) for k in range(4)]

            # ---- pool0: 1x1 conv over full res, bn, relu6, mean, broadcast
            t0 = tp.tile([128, L], fp32, tag="t0", bufs=1)
            for th in range(2):
                acc = ps.tile([128, 512], fp32, tag="mm")
                for k in range(4):
                    te.matmul(acc[:], wpt[k][:], xc[k][:, th * 512:(th + 1) * 512],
                              start=(k == 0), stop=(k == 3))
                se.activation(t0[:, th * 512:(th + 1) * 512], acc[:], AF.Identity,
                              bias=pbb[:, 0:1], scale=pbs[:, 0:1])
            t0b = tp.tile([128, L], fp32, tag="t0b", bufs=1)
            relu6(t0b[:], t0[:])
            mean = tp.tile([128, 1], fp32, tag="mean")
            ve.tensor_reduce(mean[:], t0b[:], mybir.AxisListType.X, OP.add)
            means = tp.tile([128, 1], fp32, tag="means")
            ve.tensor_scalar_mul(means[:], mean[:], 1.0 / L)
            se.activation(xc[4][:], ones[:], AF.Copy, scale=means[:, 0:1])

            # ---- pools 1..3
            for i, p in enumerate(POOL_SCALES[1:], start=1):
                hb = _bin_ranges(H, p)
                wb = _bin_ranges(W, p)
                pp = p * p
                scpl = tp.tile([128, pp], bf16, tag="scpl")
                nc.sync.dma_start(scpl[:], di[f"scpl_{p}"][:])
                nk = (pp + 127) // 128
                w2t = [wp.tile([128, 1024], bf16, tag="w2", name=f"w2t{kk}") for kk in range(nk)]
                for kk in range(nk):
                    r0 = kk * 128
                    r1 = min(pp, r0 + 128)
                    nc.sync.dma_start(w2t[kk][0:r1 - r0, :], di[f"w2_{p}"][r0:r1, :])

                conv_ps = pst.tile([128, pp], fp32, tag="cps")
                for k in range(4):
                    # pool w then h via strided reduces, per c-tile
                    pw = tp.tile([128, 32 * p], fp32, tag="pw")  # [h, q]
                    xv = xc[k][:].rearrange("c (h w) -> c h w", w=W)
                    pwv = pw[:].rearrange("c (h q) -> c h q", q=p)
                    for qi, (s, e) in enumerate(wb):
                        ve.tensor_reduce(pwv[:, :, qi:qi + 1], xv[:, :, s:e],
                                         mybir.AxisListType.X, OP.add)
                    pooled = tp.tile([128, pp], fp32, tag="pooled")
                    pldv = pooled[:].rearrange("c (ph q) -> c ph q", q=p)
                    pwT = pw[:].rearrange("c (h q) -> c q h", q=p)  # strided view
                    for pi, (s, e) in enumerate(hb):
                        ve.tensor_reduce(pldv[:, pi, :].unsqueeze(1), pwT[:, :, s:e],
                                         mybir.AxisListType.X, OP.add)
                    pooled_s = tp.tile([128, pp], bf16, tag="pooled_s")
                    ve.tensor_tensor(pooled_s[:], pooled[:], scpl[:], OP.mult)
                    te.matmul(conv_ps[:], wpt[i * 4 + k][:], pooled_s[:],
                              start=(k == 0), stop=(k == 3))
                tbn = tp.tile([128, pp], fp32, tag="tbn")
                se.activation(tbn[:], conv_ps[:], AF.Identity,
                              bias=pbb[:, i:i + 1], scale=pbs[:, i:i + 1])
                tr6 = tp.tile([128, pp], bf16, tag="tr6")
                relu6(tr6[:], tbn[:])
                # transpose [128, pp] -> [pp, 128]
                tT = [tp.tile([128, 128], bf16, tag="tT", name=f"tT{kk}") for kk in range(nk)]
                for kk in range(nk):
                    r0 = kk * 128
                    r1 = min(pp, r0 + 128)
                    tps_ = pst.tile([128, 128], bf16, tag="trp")
                    te.transpose(tps_[0:r1 - r0, :], tr6[:, r0:r1], ident[:])
                    se.activation(tT[kk][0:r1 - r0, :], tps_[0:r1 - r0, :], AF.Copy)
                for th in range(2):
                    rs = ps.tile([128, 512], fp32, tag="mm")
                    for kk in range(nk):
                        r0 = kk * 128
                        r1 = min(pp, r0 + 128)
                        te.matmul(rs[:], tT[kk][0:r1 - r0, :],
                                  w2t[kk][0:r1 - r0, th * 512:(th + 1) * 512],
                                  start=(kk == 0), stop=(kk == nk - 1))
                    se.activation(xc[4 + i][:, th * 512:(th + 1) * 512], rs[:], AF.Copy)

        if debug_taps:
            for k in range(8):
                cp = cst.tile([128, L], fp32, tag="dbgcp")
                se.activation(cp[:], xc[k][:], AF.Copy)
                nc.sync.dma_start(taps["t_xc"][k * 128:(k + 1) * 128, :], cp[:])

        # ============ PHASE 2+3: in_proj, conv1d+silu ============
        xmc = [None] * NT
        z = [None] * NT

        with ExitStack() as p2:
            ps = p2.enter_context(tc.tile_pool(name="ps2", bufs=4, space="PSUM"))
            wp = p2.enter_context(tc.tile_pool(name="wp2", bufs=10))
            xmp = p2.enter_context(tc.tile_pool(name="xmp", bufs=4))
            ac = p2.enter_context(tc.tile_pool(name="ac2", bufs=4))

            for m in range(2 * NT):  # 0..15 xm, 16..31 z
                wpan = wp.tile([128, 1024], bf16, tag="w_in")
                nc.sync.dma_start(wpan[:], di["in_proj_pk"][m])
                wt = [wpan[:, k * 128:(k + 1) * 128] for k in range(8)]
                if m < NT:
                    xm_t = xmp.tile([128, L + PAD], bf16, tag="xm")
                    ve.memset(xm_t[:, 0:PAD], 0.0)
                    dst = xm_t
                    off = PAD
                else:
                    z[m - NT] = zygp.tile([128, L], bf16, tag="zyg", name=f"z{m}")
                    dst = z[m - NT]
                    off = 0
                for th in range(2):
                    acc = ps.tile([128, 512], fp32, tag="mm")
                    for k in range(8):
                        te.matmul(acc[:], wt[k][:], xc[k][:, th * 512:(th + 1) * 512],
                                  start=(k == 0), stop=(k == 7))
                    # z rows get silu applied here (Silu table active in this
                    # phase) so the scan phase only needs the Exp/Ln table.
                    se.activation(dst[:, off + th * 512: off + (th + 1) * 512],
                                  acc[:], AF.Copy if m < NT else AF.Silu)
                if m < NT:
                    # causal depthwise conv1d + silu
                    a0 = ac.tile([128, L], bf16, tag="cacc")
                    a1 = ac.tile([128, L], bf16, tag="cacc")
                    ve.tensor_scalar_mul(a0[:], xm_t[:, 0:L], conv_w[:, m * 4:m * 4 + 1])
                    ve.scalar_tensor_tensor(a1[:], xm_t[:, 1:1 + L],
                                            conv_w[:, m * 4 + 1:m * 4 + 2], a0[:],
                                            OP.mult, OP.add)
                    ve.scalar_tensor_tensor(a0[:], xm_t[:, 2:2 + L],
                                            conv_w[:, m * 4 + 2:m * 4 + 3], a1[:],
                                            OP.mult, OP.add)
                    ve.scalar_tensor_tensor(a1[:], xm_t[:, 3:3 + L],
                                            conv_w[:, m * 4 + 3:m * 4 + 4], a0[:],
                                            OP.mult, OP.add)
                    xmc[m] = xmcp.tile([128, L], bf16, tag="xmc", name=f"xmc{m}")
                    se.activation(xmc[m][:], a1[:], AF.Silu, bias=conv_b[:, m:m + 1])

        if debug_taps:
            for k in range(NT):
                cp = cst.tile([128, L], fp32, tag="dbgcp")
                se.activation(cp[:], xmc[k][:], AF.Copy)
                nc.sync.dma_start(taps["t_xmc"][k * 128:(k + 1) * 128, :], cp[:])

        pxc.close()  # frees xc + ones

        # ============ PHASE 4: x_proj ============
        xdbl_sb = cst.tile([96, L], bf16, tag="xdbl")
        with ExitStack() as p4:
            psb = p4.enter_context(tc.tile_pool(name="ps4", bufs=2, space="PSUM"))
            wp = p4.enter_context(tc.tile_pool(name="wp4", bufs=4))
            xd_ps = psb.tile([128, L], fp32, tag="xd")
            xpan = wp.tile([128, 16 * 96], bf16, tag="w_xp")
            nc.sync.dma_start(xpan[:], di["x_proj_pk"][:])
            for k in range(NT):
                for th in range(2):
                    te.matmul(xd_ps[0:96, th * 512:(th + 1) * 512],
                              xpan[:, k * 96:(k + 1) * 96],
                              xmc[k][:, th * 512:(th + 1) * 512],
                              start=(k == 0), stop=(k == NT - 1))
            se.activation(xdbl_sb[:], xd_ps[0:96, :], AF.Copy)

        if debug_taps:
            cp = cst.tile([96, L], fp32, tag="dbgcp96")
            se.activation(cp[:], xdbl_sb[:], AF.Copy)
            nc.sync.dma_start(taps["t_xdbl"][:], cp[:])

        # ============ PHASE 5: broadcasts + scan + gate ============
        bcp = pmid.enter_context(tc.tile_pool(name="bcp", bufs=32))
        Bb = [bcp.tile([128, L], bf16, tag="bc", name=f"Bb{n}") for n in range(D_STATE)]
        Cb = [bcp.tile([128, L], bf16, tag="bc", name=f"Cb{n}") for n in range(D_STATE)]
        bsp = pmid.enter_context(tc.tile_pool(name="bsp", bufs=4))
        for n in range(2 * D_STATE):
            dst = Bb[n] if n < D_STATE else Cb[n - D_STATE]
            stg = bsp.tile([1, L], bf16, tag="bstg", name=f"bstg{n}")
            nc.sync.dma_start(stg[0:1, :], xdbl_sb[DT_RANK + n:DT_RANK + n + 1, :])
            ge.partition_broadcast(dst[:], stg[0:1, :])

        yg = [None] * NT
        with ExitStack() as p5:
            psb = p5.enter_context(tc.tile_pool(name="ps5", bufs=2, space="PSUM"))
            sp = p5.enter_context(tc.tile_pool(name="sp5", bufs=2))
            spb = p5.enter_context(tc.tile_pool(name="spb5", bufs=2))
            ysp = p5.enter_context(tc.tile_pool(name="ysp", bufs=1))

            for k in range(NT):
                dt_ps = psb.tile([128, L], fp32, tag="dtps")
                for th in range(2):
                    te.matmul(dt_ps[:, th * 512:(th + 1) * 512],
                              dt_proj_wT[:, k * 128:(k + 1) * 128],
                              xdbl_sb[0:DT_RANK, th * 512:(th + 1) * 512],
                              start=True, stop=True)
                dt_e = sp.tile([128, L], bf16, tag="dte", bufs=2)
                se.activation(dt_e[:], dt_ps[:], AF.Exp, bias=dt_bias[:, k:k + 1])
                dt_t = sp.tile([128, L], bf16, tag="dt")
                se.activation(dt_t[:], dt_e[:], AF.Ln, bias=1.0)
                if debug_taps:
                    cp = cst.tile([128, L], fp32, tag="dbgcp")
                    se.activation(cp[:], dt_t[:], AF.Copy)
                    nc.sync.dma_start(taps["t_dt"][k * 128:(k + 1) * 128, :], cp[:])
                w_t = sp.tile([128, L], bf16, tag="wt", bufs=1)
                ve.tensor_tensor(w_t[:], dt_t[:], xmc[k][:], OP.mult)
                ys = ysp.tile([128, D_STATE * L], bf16, tag="ys")
                for n in range(D_STATE):
                    a_t = spb.tile([128, L], bf16, tag="a")
                    se.activation(a_t[:], dt_t[:], AF.Exp,
                                  scale=A_sb[:, k * D_STATE + n:k * D_STATE + n + 1])
                    b_t = spb.tile([128, L], bf16, tag="b", bufs=3)
                    # balance elementwise muls across DVE and gpsimd
                    (ge if GP_B(n) else ve).tensor_tensor(b_t[:], w_t[:], Bb[n][:], OP.mult)
                    h_t = spb.tile([128, L], bf16, tag="h", bufs=3)
                    ve.tensor_tensor_scan(h_t[:], a_t[:], b_t[:], 0.0, OP.mult, OP.add)
                    (ge if GP_Y(n) else ve).tensor_tensor(ys[:, n * L:(n + 1) * L], h_t[:], Cb[n][:], OP.mult)
                # in-place pairwise tree sum over the 16 slabs
                stride = 1
                cnt = D_STATE
                while cnt > 1:
                    for i in range(cnt // 2):
                        ve.tensor_tensor(
                            ys[:, (2 * i) * stride * L:((2 * i) * stride + 1) * L],
                            ys[:, (2 * i) * stride * L:((2 * i) * stride + 1) * L],
                            ys[:, (2 * i + 1) * stride * L:((2 * i + 1) * stride + 1) * L],
                            OP.add)
                    stride *= 2
                    cnt //= 2
                yfull = sp.tile([128, L], bf16, tag="yfull", bufs=1)
                ve.scalar_tensor_tensor(yfull[:], xmc[k][:], D_sb[:, k:k + 1],
                                        ys[:, 0:L], OP.mult, OP.add)
                yg[k] = zygp.tile([128, L], bf16, tag="zyg", name=f"yg{k}")
                ve.tensor_tensor(yg[k][:], yfull[:], z[k][:], OP.mult)

        if debug_taps:
            for k in range(NT):
                cp = cst.tile([128, L], fp32, tag="dbgcp")
                se.activation(cp[:], yg[k][:], AF.Copy)
                nc.sync.dma_start(taps["t_yg"][k * 128:(k + 1) * 128, :], cp[:])

        pmid.close()  # frees xmc, B/C broadcast planes

        # ============ PHASE 7: out_proj -> padded conv input ============
        cvp = ctx.enter_context(tc.tile_pool(name="cvp", bufs=8))
        convpad = [cvp.tile([128, 34 * 34], bf16, tag="cvpad", name=f"cvpad{m}") for m in range(8)]
        for m in range(8):
            ve.memset(convpad[m][:], 0.0)

        with ExitStack() as p7:
            ps = p7.enter_context(tc.tile_pool(name="ps7", bufs=4, space="PSUM"))
            wp = p7.enter_context(tc.tile_pool(name="wp7", bufs=3))
            for m in range(8):
                wpan = wp.tile([128, 2048], bf16, tag="w_op")
                nc.sync.dma_start(wpan[:], di["out_proj_pk"][m])
                for th in range(2):
                    acc = ps.tile([128, 512], fp32, tag="mm")
                    for k in range(NT):
                        te.matmul(acc[:], wpan[:, k * 128:(k + 1) * 128],
                                  yg[k][:, th * 512:(th + 1) * 512],
                                  start=(k == 0), stop=(k == NT - 1))
                    dstv = convpad[m][:].rearrange("c (h w) -> c h w", w=34)
                    se.activation(
                        dstv[:, 1 + th * 16:1 + (th + 1) * 16, 1:33],
                        acc[:].rearrange("c (h w) -> c h w", w=32), AF.Copy)

        # ============ PHASE 8: FFN 3x3 conv + bn + relu6 ============
        t2p = ctx.enter_context(tc.tile_pool(name="t2p", bufs=8))
        t2 = [t2p.tile([128, L], bf16, tag="t2", name=f"t2_{m}") for m in range(8)]
        with ExitStack() as p8:
            ps = p8.enter_context(tc.tile_pool(name="ps8", bufs=4, space="PSUM"))
            wp = p8.enter_context(tc.tile_pool(name="wp8", bufs=2))
            tmp = p8.enter_context(tc.tile_pool(name="tmp8", bufs=2))
            for m in range(8):
                wpan = wp.tile([128, 9 * 8 * 128], bf16, tag="w_ffn")
                nc.sync.dma_start(wpan[:], di["ffn_pk"][m])
                for th in range(2):
                    acc = ps.tile([128, 512], fp32, tag="mm")
                    first = True
                    for tap in range(9):
                        ky, kx = tap // 3, tap % 3
                        br = th * 16 + ky
                        for k in range(8):
                            rhs = convpad[k][:].rearrange(
                                "c (h w) -> c h w", w=34)[:, br:br + 16, kx:kx + 32]
                            woff = (tap * 8 + k) * 128
                            te.matmul(acc[:], wpan[:, woff:woff + 128], rhs,
                                      start=first, stop=(tap == 8 and k == 7))
                            first = False
                    tb = tmp.tile([128, 512], fp32, tag="tbn8")
                    se.activation(tb[:], acc[:], AF.Identity,
                                  bias=fbb[:, m:m + 1], scale=fbs[:, m:m + 1])
                    relu6(t2[m][:, th * 512:(th + 1) * 512], tb[:])

        if debug_taps:
            for k in range(8):
                cp = cst.tile([128, L], fp32, tag="dbgcp")
                se.activation(cp[:], t2[k][:], AF.Copy)
                nc.sync.dma_start(taps["t_conv"][k * 128:(k + 1) * 128, :], cp[:])

        # ============ PHASE 9/10: fc1+gelu, fc2 ============
        with ExitStack() as p9:
            ps = p9.enter_context(tc.tile_pool(name="ps9", bufs=4, space="PSUM"))
            wp = p9.enter_context(tc.tile_pool(name="wp9", bufs=10))
            gp = p9.enter_context(tc.tile_pool(name="gp9", bufs=4))
            g = [gp.tile([128, L], bf16, tag="g", name=f"g{i}") for i in range(4)]
            for mo in range(4):
                wpan = wp.tile([128, 1024], bf16, tag="w_fc1")
                nc.sync.dma_start(wpan[:], di["fc1_pk"][mo])
                for th in range(2):
                    acc = ps.tile([128, 512], fp32, tag="mm")
                    for k in range(8):
                        te.matmul(acc[:], wpan[:, k * 128:(k + 1) * 128],
                                  t2[k][:, th * 512:(th + 1) * 512],
                                  start=(k == 0), stop=(k == 7))
                    se.activation(g[mo][:, th * 512:(th + 1) * 512], acc[:], AF.Gelu)
            out_sb = gp.tile([128, L], fp32, tag="outsb", bufs=1)
            w2pan = wp.tile([128, 512], bf16, tag="w_fc2")
            nc.sync.dma_start(w2pan[:], di["fc2_pk"][:])
            for th in range(2):
                acc = ps.tile([128, 512], fp32, tag="mm")
                for k in range(4):
                    te.matmul(acc[:], w2pan[:, k * 128:(k + 1) * 128],
                              g[k][:, th * 512:(th + 1) * 512],
                              start=(k == 0), stop=(k == 3))
                se.activation(out_sb[:, th * 512:(th + 1) * 512], acc[:], AF.Copy)
            nc.sync.dma_start(out_dram[:], out_sb[:])

    nc.compile()
    return nc


_PROGRAM_CACHE = {}


def run(inputs, debug_taps=False, trace=False):
    from concourse.bass_utils import run_bass_kernel_spmd

    key = debug_taps
    if key not in _PROGRAM_CACHE:
        _PROGRAM_CACHE[key] = build_program(debug_taps)
    nc = _PROGRAM_CACHE[key]

    host = _prep_host(inputs)
    x = np.asarray(inputs["x"], dtype=np.float32)
    in_maps = []
    for b in range(N_CORES):
        m = {k: v for k, v in host.items() if v is not None and not k.startswith("bins")}
        m["xin"] = np.ascontiguousarray(x[b].reshape(IN_CHS, L)).astype(BF)
        in_maps.append(m)
    res = run_bass_kernel_spmd(nc, in_maps, core_ids=list(range(N_CORES)),
                               trace=trace)
    outs = np.stack([res.results[b]["out"].reshape(128, H, W)
                     for b in range(N_CORES)])
    return outs.astype(np.float32), res


def kernel(**inputs) -> np.ndarray:
    out, _ = run(inputs)
    return out
